# revision 10
# baseline (speedup 1.0000x reference)
"""EGCL (E(n)-GNN conv layer) Trainium2 kernel.

Contract: kernel(**inputs) -> (h_out, coord_out), matching reference._egcl.

Strategy:
  * Host: index-only preprocessing. Edges sorted by destination `row`;
    destination nodes greedily packed into 160 "windows" (<=128 nodes and
    <=2048 edges each, padded with dummy edges). 20 windows per core, so all
    8 cores run one identical (SPMD) program on different data slices.
  * Device, per window: dma_gather h[row], h[col] (transposed, bf16) and
    coord rows (f32); radial via DVE broadcast-mul + reduce; edge MLP /
    coord MLP on TensorE in bf16 (f32 PSUM accumulation); segment-sum via
    one-hot selection-matrix matmuls accumulated in a PSUM bank; node MLP
    and coord update per window; outputs written per window.
  * No collectives needed: each core owns a disjoint set of destination
    nodes and all edges pointing into them.
"""
import os
import sys

sys.path.insert(0, "/opt/trn_rl_repo")

import numpy as np
import ml_dtypes

import concourse.bass as bass
import concourse.mybir as mybir
import concourse.tile as tile
from concourse import bacc
from concourse.bass_utils import run_bass_kernel_spmd
from concourse.masks import make_identity

BF16 = ml_dtypes.bfloat16

# problem constants (hardcoded per contract)
N = 20000
E = 320000
F = 128
C = 3
D = 3
CD = C * D            # 9
EIN = 2 * F + C * C   # 265
EH = 530              # edge-mlp hidden
NH = 512              # node-mlp hidden
CH = 256              # coord-mlp hidden
COORD_CLAMP = 10.0

NCORES = 8
WPC = 20              # windows per core
NWIN = NCORES * WPC   # 160
CAPE = 2048           # edge slots per window
NPW = 128             # node slots per window
EPC = WPC * CAPE      # 40960 edge slots per core
NSPC = WPC * NPW      # 2560 node slots per core
NT4 = CAPE // 512     # 4  (512-edge matmul tiles per window)
NJ16 = CAPE // 128    # 16 (128-edge blocks per window)

F32 = mybir.dt.float32
BF = mybir.dt.bfloat16
I16 = mybir.dt.int16
I32 = mybir.dt.int32
AF = mybir.ActivationFunctionType
OP = mybir.AluOpType


# ---------------------------------------------------------------------------
# device program
# ---------------------------------------------------------------------------

def build_nc():
    nc = bacc.Bacc("TRN2", target_bir_lowering=False, debug=False,
                   num_devices=NCORES)

    # DRAM tensors
    hW = nc.dram_tensor("hW", [N, F], BF, kind="ExternalInput")
    coordp = nc.dram_tensor("coordp", [N, 64], F32, kind="ExternalInput")
    gidx_row = nc.dram_tensor("gidx_row", [128, EPC // 16], I16, kind="ExternalInput")
    gidx_col = nc.dram_tensor("gidx_col", [128, EPC // 16], I16, kind="ExternalInput")
    rel = nc.dram_tensor("rel", [128, EPC // 128], F32, kind="ExternalInput")
    hT = nc.dram_tensor("hT", [F, NSPC], BF, kind="ExternalInput")
    hTf = nc.dram_tensor("hTf", [F, NSPC], F32, kind="ExternalInput")
    coordn = nc.dram_tensor("coordn", [NPW, WPC, CD], F32, kind="ExternalInput")
    invdeg = nc.dram_tensor("invdeg", [NPW, WPC], F32, kind="ExternalInput")

    eW1a = nc.dram_tensor("eW1a", [128, 544], BF, kind="ExternalInput")
    eW1b = nc.dram_tensor("eW1b", [128, 544], BF, kind="ExternalInput")
    eW1c = nc.dram_tensor("eW1c", [16, 544], BF, kind="ExternalInput")
    eW2m = nc.dram_tensor("eW2m", [128, 4, 128], BF, kind="ExternalInput")
    eW2t = nc.dram_tensor("eW2t", [32, 128], BF, kind="ExternalInput")
    cW1 = nc.dram_tensor("cW1", [128, 256], BF, kind="ExternalInput")
    cW2m = nc.dram_tensor("cW2m", [128, 2, 8], BF, kind="ExternalInput")
    nW1m = nc.dram_tensor("nW1m", [128, 2, 4, 128], BF, kind="ExternalInput")
    nW2m = nc.dram_tensor("nW2m", [128, 4, 128], BF, kind="ExternalInput")
    eb1 = nc.dram_tensor("eb1", [128, 5], F32, kind="ExternalInput")
    eb2 = nc.dram_tensor("eb2", [128, 1], F32, kind="ExternalInput")
    cb1 = nc.dram_tensor("cb1", [128, 2], F32, kind="ExternalInput")
    nb1 = nc.dram_tensor("nb1", [128, 4], F32, kind="ExternalInput")
    nb2 = nc.dram_tensor("nb2", [128, 1], F32, kind="ExternalInput")

    houtT = nc.dram_tensor("houtT", [F, NSPC], F32, kind="ExternalOutput")
    coordo = nc.dram_tensor("coordo", [NPW, WPC, CD], F32, kind="ExternalOutput")

    with tile.TileContext(nc) as tc:
        with (
            tc.tile_pool(name="const", bufs=1) as cp,
            tc.tile_pool(name="work", bufs=2) as wp,
            tc.tile_pool(name="psum", bufs=2, space="PSUM") as pp,
            tc.tile_pool(name="psA", bufs=1, space="PSUM") as pa,
        ):
            # ---- static loads ----
            def load(t, shape, dt, src):
                s = cp.tile(shape, dt, tag=t, name=t)
                nc.sync.dma_start(s[:], src[:])
                return s

            gidxr_sb = load("gr", [128, EPC // 16], I16, gidx_row)
            gidxc_sb = load("gc", [128, EPC // 16], I16, gidx_col)
            rel_sb = load("rel", [128, EPC // 128], F32, rel)
            hT_sb = load("hT", [F, NSPC], BF, hT)
            hTf_sb = load("hTf", [F, NSPC], F32, hTf)
            coordn_sb = load("cn", [NPW, WPC, CD], F32, coordn)
            invdeg_sb = load("ivd", [NPW, WPC], F32, invdeg)
            eW1a_sb = load("w1a", [128, 544], BF, eW1a)
            eW1b_sb = load("w1b", [128, 544], BF, eW1b)
            eW1c_sb = load("w1c", [16, 544], BF, eW1c)
            eW2m_sb = load("w2m", [128, 4, 128], BF, eW2m)
            eW2t_sb = load("w2t", [32, 128], BF, eW2t)
            cW1_sb = load("cw1", [128, 256], BF, cW1)
            cW2m_sb = load("cw2", [128, 2, 8], BF, cW2m)
            nW1m_sb = load("nw1", [128, 2, 4, 128], BF, nW1m)
            nW2m_sb = load("nw2", [128, 4, 128], BF, nW2m)
            eb1_sb = load("eb1", [128, 5], F32, eb1)
            eb2_sb = load("eb2", [128, 1], F32, eb2)
            cb1_sb = load("cb1", [128, 2], F32, cb1)
            nb1_sb = load("nb1", [128, 4], F32, nb1)
            nb2_sb = load("nb2", [128, 1], F32, nb2)

            iota_i = cp.tile([128, 128], I32, tag="iota_i")
            nc.gpsimd.iota(iota_i[:], pattern=[[1, 128]], base=0,
                           channel_multiplier=0)
            iota_f = cp.tile([128, 128], F32, tag="iota_f")
            nc.vector.tensor_copy(iota_f[:], iota_i[:])
            ident = cp.tile([128, 128], BF, tag="ident")
            make_identity(nc, ident[:])

            for w in range(WPC):
                # per-window PSUM accumulators (separate banks: one open
                # accumulation group per bank)
                aggp = pa.tile([128, 128], F32, tag="aggp")
                aggcp = pa.tile([128, 16], F32, tag="aggcp")

                for t4 in range(NT4):
                    b512 = NT4 * w + t4
                    isl = slice(32 * b512, 32 * b512 + 32)
                    # ---- gathers for this 512-edge tile ----
                    g_row = wp.tile([128, 1, 512], BF, tag="grow")
                    nc.gpsimd.dma_gather(
                        out_ap=g_row[:], in_ap=hW[:], idxs_ap=gidxr_sb[:, isl],
                        num_idxs=512, num_idxs_reg=512, elem_size=F,
                        transpose=True)
                    g_col = wp.tile([128, 1, 512], BF, tag="gcol")
                    nc.gpsimd.dma_gather(
                        out_ap=g_col[:], in_ap=hW[:], idxs_ap=gidxc_sb[:, isl],
                        num_idxs=512, num_idxs_reg=512, elem_size=F,
                        transpose=True)
                    c_row = wp.tile([128, 4, 64], F32, tag="crow")
                    nc.gpsimd.dma_gather(
                        out_ap=c_row[:], in_ap=coordp[:], idxs_ap=gidxr_sb[:, isl],
                        num_idxs=512, num_idxs_reg=512, elem_size=64,
                        transpose=False)
                    c_col = wp.tile([128, 4, 64], F32, tag="ccol")
                    nc.gpsimd.dma_gather(
                        out_ap=c_col[:], in_ap=coordp[:], idxs_ap=gidxc_sb[:, isl],
                        num_idxs=512, num_idxs_reg=512, elem_size=64,
                        transpose=False)

                    # ---- coord_diff & radial ----
                    cd = wp.tile([128, 4, CD], F32, tag="cd")
                    nc.vector.tensor_tensor(
                        out=cd[:], in0=c_row[:, :, 0:CD], in1=c_col[:, :, 0:CD],
                        op=OP.subtract)
                    t27 = wp.tile([128, 4, 27], F32, tag="t27")
                    cd4 = cd[:].rearrange("p t (c d) -> p t c d", c=3)
                    in0 = cd4.unsqueeze(3).broadcast_to([128, 4, 3, 3, 3])
                    in1 = cd4.unsqueeze(2).broadcast_to([128, 4, 3, 3, 3])
                    nc.vector.tensor_tensor(
                        out=t27[:].rearrange("p t (c f d) -> p t c f d", c=3, f=3),
                        in0=in0, in1=in1, op=OP.mult)
                    radial = wp.tile([128, 4, CD], F32, tag="radial")
                    nc.vector.tensor_reduce(
                        out=radial[:], in_=t27[:].rearrange("p t (x d) -> p t x d", d=3),
                        axis=mybir.AxisListType.X, op=OP.add)
                    rad_bf = wp.tile([128, 4, 16], BF, tag="radbf")
                    nc.vector.memset(rad_bf[:], 0)
                    nc.vector.tensor_copy(rad_bf[:, :, 0:CD], radial[:])
                    radT = wp.tile([16, 4, 128], BF, tag="radT")
                    for j in range(4):
                        tp = pp.tile([16, 128], BF, tag="tps")
                        nc.tensor.transpose(tp[:], rad_bf[:, j, :], ident[:])
                        nc.vector.tensor_copy(radT[:, j, :], tp[:])
                    # ---- edge MLP layer 1 ----
                    hid = []
                    for m in range(5):
                        mw = 128 if m < 4 else 32
                        msl = slice(128 * m, 128 * m + mw)
                        hp = pp.tile([mw, 512], F32, tag="mm512")
                        nc.tensor.matmul(hp[:], eW1a_sb[:, msl],
                                         g_row[:, 0, :], start=True, stop=False)
                        nc.tensor.matmul(hp[:], eW1b_sb[:, msl],
                                         g_col[:, 0, :], start=False, stop=False)
                        nc.tensor.matmul(
                            hp[:], eW1c_sb[:, msl],
                            radT[:].rearrange("p a b -> p (a b)"),
                            start=False, stop=True)
                        hs = wp.tile([mw, 512], BF, tag=f"hid{m}")
                        nc.scalar.activation(hs[:], hp[:], AF.Silu,
                                             bias=eb1_sb[0:mw, m:m + 1])
                        hid.append(hs)
                    # ---- edge MLP layer 2 ----
                    efp = pp.tile([128, 512], F32, tag="mm512")
                    for m in range(4):
                        nc.tensor.matmul(efp[:], eW2m_sb[:, m, :], hid[m][:],
                                         start=(m == 0), stop=False)
                    nc.tensor.matmul(efp[:], eW2t_sb[:], hid[4][:],
                                     start=False, stop=True)
                    ef = wp.tile([128, 512], BF, tag="ef")
                    nc.scalar.activation(ef[:], efp[:], AF.Silu,
                                         bias=eb2_sb[:, 0:1])
                    # ---- coord MLP ----
                    h2 = []
                    for m in range(2):
                        h2p = pp.tile([128, 512], F32, tag="mm512")
                        nc.tensor.matmul(h2p[:], cW1_sb[:, 128 * m:128 * m + 128],
                                         ef[:], start=True, stop=True)
                        h2s = wp.tile([128, 512], BF, tag=f"h2{m}")
                        nc.scalar.activation(h2s[:], h2p[:], AF.Silu,
                                             bias=cb1_sb[:, m:m + 1])
                        h2.append(h2s)
                    php = pp.tile([8, 512], F32, tag="ps2")
                    nc.tensor.matmul(php[:], cW2m_sb[:, 0, :], h2[0][:],
                                     start=True, stop=False)
                    nc.tensor.matmul(php[:], cW2m_sb[:, 1, :], h2[1][:],
                                     start=False, stop=True)
                    phT = wp.tile([8, 512], BF, tag="phT")
                    nc.vector.tensor_copy(phT[:], php[:])
                    # transpose phi -> [128, 4, 8]
                    phi = wp.tile([128, 4, 8], F32, tag="phi")
                    for j in range(4):
                        pt = pp.tile([128, 8], BF, tag="tps")
                        nc.tensor.transpose(pt[:], phT[:, 128 * j:128 * j + 128],
                                            ident[0:8, 0:8])
                        nc.vector.tensor_copy(phi[:, j, :], pt[:])
                    # trans = coord_diff * phi (broadcast over d)
                    trans = wp.tile([128, 4, CD], BF, tag="trans")
                    cdv = cd[:].rearrange("p t (c d) -> p t c d", c=3)
                    phv = phi[:, :, 0:3].unsqueeze(3).broadcast_to([128, 4, 3, 3])
                    nc.vector.tensor_tensor(
                        out=trans[:].rearrange("p t (c d) -> p t c d", c=3),
                        in0=cdv, in1=phv, op=OP.mult)
                    # ---- aggregation ----
                    for j in range(4):
                        g16 = 4 * t4 + j
                        ep = pp.tile([128, 128], BF, tag="tps")
                        nc.tensor.transpose(ep[:], ef[:, 128 * j:128 * j + 128],
                                            ident[:])
                        eft = wp.tile([128, 128], BF, tag="eft")
                        nc.vector.tensor_copy(eft[:], ep[:])
                        smat = wp.tile([128, 128], BF, tag="smat")
                        nc.vector.tensor_scalar(
                            out=smat[:], in0=iota_f[:],
                            scalar1=rel_sb[:, NJ16 * w + g16:NJ16 * w + g16 + 1],
                            scalar2=None, op0=OP.is_equal)
                        nc.tensor.matmul(aggp[:, 0:128], smat[:], eft[:],
                                         start=(g16 == 0), stop=(g16 == NJ16 - 1),
                                         skip_group_check=True)
                        nc.tensor.matmul(aggcp[:, 0:9], smat[:],
                                         trans[:, j, :],
                                         start=(g16 == 0), stop=(g16 == NJ16 - 1),
                                         skip_group_check=True)

                # ---- node update for this window ----
                nsl = slice(128 * w, 128 * w + 128)
                agg_sb = wp.tile([128, 137], F32, tag="aggsb")
                nc.vector.tensor_copy(agg_sb[:, 0:128], aggp[:, 0:128])
                nc.vector.tensor_copy(agg_sb[:, 128:137], aggcp[:, 0:9])
                aggh_bf = wp.tile([128, 128], BF, tag="agghbf")
                nc.vector.tensor_copy(aggh_bf[:], agg_sb[:, 0:128])
                atp = pp.tile([128, 128], BF, tag="tps")
                nc.tensor.transpose(atp[:], aggh_bf[:], ident[:])
                aggT = wp.tile([128, 128], BF, tag="aggT")
                nc.vector.tensor_copy(aggT[:], atp[:])
                h3 = []
                for m in range(4):
                    h3p = pp.tile([128, 128], F32, tag="ps2")
                    nc.tensor.matmul(h3p[:], nW1m_sb[:, 0, m, :], hT_sb[:, nsl],
                                     start=True, stop=False)
                    nc.tensor.matmul(h3p[:], nW1m_sb[:, 1, m, :], aggT[:],
                                     start=False, stop=True)
                    h3s = wp.tile([128, 128], BF, tag=f"h3{m}")
                    nc.scalar.activation(h3s[:], h3p[:], AF.Silu,
                                         bias=nb1_sb[:, m:m + 1])
                    h3.append(h3s)
                h4p = pp.tile([128, 128], F32, tag="ps2")
                for m in range(4):
                    nc.tensor.matmul(h4p[:], nW2m_sb[:, m, :], h3[m][:],
                                     start=(m == 0), stop=(m == 3))
                h4s = wp.tile([128, 128], F32, tag="h4s")
                nc.scalar.activation(h4s[:], h4p[:], AF.Silu,
                                     bias=nb2_sb[:, 0:1])
                hout = wp.tile([128, 128], F32, tag="hout")
                nc.vector.tensor_tensor(out=hout[:], in0=h4s[:],
                                        in1=hTf_sb[:, nsl], op=OP.add)
                nc.sync.dma_start(houtT[:, nsl], hout[:])
                # coord update
                aggc = wp.tile([128, CD], F32, tag="aggc")
                nc.vector.tensor_scalar(
                    out=aggc[:], in0=agg_sb[:, 128:137],
                    scalar1=invdeg_sb[:, w:w + 1], scalar2=None, op0=OP.mult)
                nc.vector.tensor_scalar(
                    out=aggc[:], in0=aggc[:], scalar1=COORD_CLAMP,
                    scalar2=-COORD_CLAMP, op0=OP.min, op1=OP.max)
                co = wp.tile([128, CD], F32, tag="co")
                nc.vector.tensor_tensor(out=co[:], in0=aggc[:],
                                        in1=coordn_sb[:, w, :], op=OP.add)
                nc.sync.dma_start(coordo[:, w, :], co[:])

    nc.compile()
    return nc


# ---------------------------------------------------------------------------
# host-side prep / post
# ---------------------------------------------------------------------------

def _pack_windows(row, deg):
    """Greedy pack nodes (in order) into windows with <=NPW nodes and
    <=CAPE edges. Returns list of (node_start, n_nodes, edge_start, n_edges)
    where edge_start indexes the row-sorted edge order."""
    windows = []
    cum = np.concatenate([[0], np.cumsum(deg)])
    nstart = 0
    cur_nodes = 0
    cur_edges = 0
    for n in range(N):
        d = int(deg[n])
        if cur_nodes + 1 > NPW or cur_edges + d > CAPE:
            windows.append((nstart, cur_nodes, int(cum[nstart]), cur_edges))
            nstart = n
            cur_nodes = 0
            cur_edges = 0
        cur_nodes += 1
        cur_edges += d
    windows.append((nstart, cur_nodes, int(cum[nstart]), cur_edges))
    assert len(windows) <= NWIN, f"packing needs {len(windows)} windows > {NWIN}"
    while len(windows) < NWIN:
        windows.append((0, 0, 0, 0))
    return windows


def prepare(inputs):
    h = np.ascontiguousarray(np.asarray(inputs["h"], dtype=np.float32))
    ei = np.asarray(inputs["edge_index"])
    coord = np.asarray(inputs["coord"], dtype=np.float32)
    row = ei[0].astype(np.int64)
    col = ei[1].astype(np.int64)

    deg = np.bincount(row, minlength=N)
    order = np.argsort(row, kind="stable")
    windows = _pack_windows(row, deg)

    # edge slot arrays (global, then sliced per core)
    slot_row = np.zeros(NWIN * CAPE, dtype=np.int16)
    slot_col = np.zeros(NWIN * CAPE, dtype=np.int16)
    slot_rel = np.full(NWIN * CAPE, 255.0, dtype=np.float32)
    slot2node = np.full(NWIN * NPW, -1, dtype=np.int64)
    for w, (ns, nn, es, ne) in enumerate(windows):
        if ne:
            eidx = order[es:es + ne]
            sl = slice(w * CAPE, w * CAPE + ne)
            slot_row[sl] = row[eidx].astype(np.int16)
            slot_col[sl] = col[eidx].astype(np.int16)
            slot_rel[sl] = (row[eidx] - ns).astype(np.float32)
        if nn:
            slot2node[w * NPW:w * NPW + nn] = ns + np.arange(nn)

    hW = h.astype(BF16)
    coordp = np.zeros((N, 64), dtype=np.float32)
    coordp[:, :CD] = coord.reshape(N, CD)

    # weights
    def bf(x):
        return np.ascontiguousarray(np.asarray(x, dtype=np.float32)).astype(BF16)

    eW1 = np.asarray(inputs["eW1"], dtype=np.float32)
    eW1a = np.zeros((128, 544), np.float32); eW1a[:, :EH] = eW1[0:128]
    eW1b = np.zeros((128, 544), np.float32); eW1b[:, :EH] = eW1[128:256]
    eW1c = np.zeros((16, 544), np.float32); eW1c[:CD, :EH] = eW1[256:265]
    eW2 = np.asarray(inputs["eW2"], dtype=np.float32)
    eW2m = eW2[:512].reshape(4, 128, 128).transpose(1, 0, 2)
    eW2t = np.zeros((32, 128), np.float32); eW2t[:EH - 512] = eW2[512:EH]
    cW1 = np.asarray(inputs["cW1"], dtype=np.float32)
    cW2 = np.asarray(inputs["cW2"], dtype=np.float32)
    cW2m = np.zeros((128, 2, 8), np.float32)
    cW2m[:, :, 0:3] = cW2.reshape(2, 128, 3).transpose(1, 0, 2)
    nW1 = np.asarray(inputs["nW1"], dtype=np.float32)
    nW1m = nW1.reshape(2, 128, 4, 128).transpose(1, 0, 2, 3)
    nW2 = np.asarray(inputs["nW2"], dtype=np.float32)
    nW2m = nW2.reshape(4, 128, 128).transpose(1, 0, 2)

    def bias_fold(b, cols):
        z = np.zeros(128 * cols, np.float32)
        z[:b.shape[0]] = np.asarray(b, dtype=np.float32)
        return z.reshape(cols, 128).T.copy()

    eb1 = bias_fold(inputs["eb1"], 5)
    eb2 = bias_fold(inputs["eb2"], 1)
    cb1 = bias_fold(inputs["cb1"], 2)
    nb1 = bias_fold(inputs["nb1"], 4)
    nb2 = bias_fold(inputs["nb2"], 1)

    shared = {
        "hW": hW, "coordp": coordp,
        "eW1a": bf(eW1a), "eW1b": bf(eW1b), "eW1c": bf(eW1c),
        "eW2m": bf(eW2m), "eW2t": bf(eW2t),
        "cW1": bf(cW1), "cW2m": bf(cW2m),
        "nW1m": bf(nW1m), "nW2m": bf(nW2m),
        "eb1": eb1, "eb2": eb2, "cb1": cb1, "nb1": nb1, "nb2": nb2,
    }

    in_maps = []
    coord9 = coord.reshape(N, CD)
    invdeg_full = (1.0 / np.maximum(deg, 1)).astype(np.float32)
    for c in range(NCORES):
        esl = slice(c * EPC, (c + 1) * EPC)
        gr = np.tile(slot_row[esl].reshape(EPC // 16, 16).T, (8, 1)).copy()
        gc = np.tile(slot_col[esl].reshape(EPC // 16, 16).T, (8, 1)).copy()
        rl = slot_rel[esl].reshape(EPC // 128, 128).T.copy()
        s2n = slot2node[c * NSPC:(c + 1) * NSPC]
        valid = s2n >= 0
        idx0 = np.where(valid, s2n, 0)
        hT_c = (h[idx0] * valid[:, None]).T.copy()
        cn = (coord9[idx0] * valid[:, None]).reshape(WPC, NPW, CD)
        cn = np.ascontiguousarray(cn.transpose(1, 0, 2))
        ivd = (invdeg_full[idx0] * valid).reshape(WPC, NPW).T.copy()
        in_maps.append({
            **shared,
            "gidx_row": gr, "gidx_col": gc, "rel": rl,
            "hT": hT_c.astype(BF16), "hTf": hT_c.astype(np.float32),
            "coordn": cn.astype(np.float32), "invdeg": ivd.astype(np.float32),
        })

    meta = {"slot2node": slot2node}
    return in_maps, meta


def postprocess(results, meta):
    slot2node = meta["slot2node"]
    h_out = np.zeros((N, F), dtype=np.float32)
    coord_out = np.zeros((N, CD), dtype=np.float32)
    for c in range(NCORES):
        s2n = slot2node[c * NSPC:(c + 1) * NSPC]
        valid = s2n >= 0
        ho = results[c]["houtT"].T            # [NSPC, F]
        co = np.ascontiguousarray(
            results[c]["coordo"].transpose(1, 0, 2)).reshape(NSPC, CD)
        h_out[s2n[valid]] = ho[valid]
        coord_out[s2n[valid]] = co[valid]
    return h_out, coord_out.reshape(N, C, D)


# ---------------------------------------------------------------------------
# entry point
# ---------------------------------------------------------------------------

_NC_CACHE = None


def _get_nc():
    global _NC_CACHE
    if _NC_CACHE is None:
        _NC_CACHE = build_nc()
    return _NC_CACHE


def run(inputs, **run_kwargs):
    """Returns ((h_out, coord_out), BassKernelResults)."""
    in_maps, meta = prepare(inputs)
    nc = _get_nc()
    res = run_bass_kernel_spmd(nc, in_maps, core_ids=list(range(NCORES)),
                               **run_kwargs)
    return postprocess(res.results, meta), res


def kernel(**inputs):
    out, _ = run(inputs)
    return out


# revision 11
# speedup vs baseline: 1.1960x; 1.1960x over previous
"""EGCL (E(n)-GNN conv layer) Trainium2 kernel.

Contract: kernel(**inputs) -> (h_out, coord_out), matching reference._egcl.

Strategy:
  * Host: index-only preprocessing. Edges sorted by destination `row`;
    destination nodes greedily packed into 160 "windows" (<=128 nodes and
    <=2048 edges each, padded with dummy edges). 20 windows per core, so all
    8 cores run one identical (SPMD) program on different data slices.
  * Device, per window: dma_gather h[row], h[col] (transposed, bf16) and
    coord rows (f32); radial via DVE broadcast-mul + reduce; edge MLP /
    coord MLP on TensorE in bf16 (f32 PSUM accumulation); segment-sum via
    one-hot selection-matrix matmuls accumulated in a PSUM bank; node MLP
    and coord update per window; outputs written per window.
  * No collectives needed: each core owns a disjoint set of destination
    nodes and all edges pointing into them.
"""
import os
import sys

sys.path.insert(0, "/opt/trn_rl_repo")

import numpy as np
import ml_dtypes

import concourse.bass as bass
import concourse.mybir as mybir
import concourse.tile as tile
from concourse import bacc
from concourse.bass_utils import run_bass_kernel_spmd
from concourse.masks import make_identity

BF16 = ml_dtypes.bfloat16

# problem constants (hardcoded per contract)
N = 20000
E = 320000
F = 128
C = 3
D = 3
CD = C * D            # 9
EIN = 2 * F + C * C   # 265
EH = 530              # edge-mlp hidden
NH = 512              # node-mlp hidden
CH = 256              # coord-mlp hidden
COORD_CLAMP = 10.0

NCORES = 8
WPC = 20              # windows per core
NWIN = NCORES * WPC   # 160
CAPE = 2048           # edge slots per window
NPW = 128             # node slots per window
EPC = WPC * CAPE      # 40960 edge slots per core
NSPC = WPC * NPW      # 2560 node slots per core
NT4 = CAPE // 512     # 4  (512-edge matmul tiles per window)
NJ16 = CAPE // 128    # 16 (128-edge blocks per window)

F32 = mybir.dt.float32
BF = mybir.dt.bfloat16
I16 = mybir.dt.int16
I32 = mybir.dt.int32
AF = mybir.ActivationFunctionType
OP = mybir.AluOpType


# ---------------------------------------------------------------------------
# device program
# ---------------------------------------------------------------------------

def build_nc():
    nc = bacc.Bacc("TRN2", target_bir_lowering=False, debug=False,
                   num_devices=NCORES)

    # DRAM tensors
    hW = nc.dram_tensor("hW", [N, F], BF, kind="ExternalInput")
    coordp = nc.dram_tensor("coordp", [N, 64], F32, kind="ExternalInput")
    gidx_row = nc.dram_tensor("gidx_row", [128, EPC // 16], I16, kind="ExternalInput")
    gidx_col = nc.dram_tensor("gidx_col", [128, EPC // 16], I16, kind="ExternalInput")
    rel = nc.dram_tensor("rel", [128, EPC // 128], F32, kind="ExternalInput")
    hT = nc.dram_tensor("hT", [F, NSPC], BF, kind="ExternalInput")
    hTf = nc.dram_tensor("hTf", [F, NSPC], F32, kind="ExternalInput")
    coordn = nc.dram_tensor("coordn", [NPW, WPC, CD], F32, kind="ExternalInput")
    invdeg = nc.dram_tensor("invdeg", [NPW, WPC], F32, kind="ExternalInput")

    eW1a = nc.dram_tensor("eW1a", [128, 544], BF, kind="ExternalInput")
    eW1b = nc.dram_tensor("eW1b", [128, 544], BF, kind="ExternalInput")
    eW1c = nc.dram_tensor("eW1c", [16, 544], BF, kind="ExternalInput")
    eW2m = nc.dram_tensor("eW2m", [128, 4, 128], BF, kind="ExternalInput")
    eW2t = nc.dram_tensor("eW2t", [32, 128], BF, kind="ExternalInput")
    cW1 = nc.dram_tensor("cW1", [128, 256], BF, kind="ExternalInput")
    cW2m = nc.dram_tensor("cW2m", [128, 2, 8], BF, kind="ExternalInput")
    nW1m = nc.dram_tensor("nW1m", [128, 2, 4, 128], BF, kind="ExternalInput")
    nW2m = nc.dram_tensor("nW2m", [128, 4, 128], BF, kind="ExternalInput")
    eb1 = nc.dram_tensor("eb1", [128, 5], F32, kind="ExternalInput")
    eb2 = nc.dram_tensor("eb2", [128, 1], F32, kind="ExternalInput")
    cb1 = nc.dram_tensor("cb1", [128, 2], F32, kind="ExternalInput")
    nb1 = nc.dram_tensor("nb1", [128, 4], F32, kind="ExternalInput")
    nb2 = nc.dram_tensor("nb2", [128, 1], F32, kind="ExternalInput")

    houtT = nc.dram_tensor("houtT", [F, NSPC], F32, kind="ExternalOutput")
    coordo = nc.dram_tensor("coordo", [NPW, WPC, CD], F32, kind="ExternalOutput")

    with tile.TileContext(nc) as tc:
        with (
            tc.tile_pool(name="const", bufs=1) as cp,
            tc.tile_pool(name="work", bufs=2) as wp,
            tc.tile_pool(name="psum", bufs=2, space="PSUM") as pp,
            tc.tile_pool(name="psA", bufs=1, space="PSUM") as pa,
        ):
            # ---- static loads ----
            def load(t, shape, dt, src):
                s = cp.tile(shape, dt, tag=t, name=t)
                nc.sync.dma_start(s[:], src[:])
                return s

            gidxr_sb = load("gr", [128, EPC // 16], I16, gidx_row)
            gidxc_sb = load("gc", [128, EPC // 16], I16, gidx_col)
            rel_sb = load("rel", [128, EPC // 128], F32, rel)
            hT_sb = load("hT", [F, NSPC], BF, hT)
            hTf_sb = load("hTf", [F, NSPC], F32, hTf)
            coordn_sb = load("cn", [NPW, WPC, CD], F32, coordn)
            invdeg_sb = load("ivd", [NPW, WPC], F32, invdeg)
            eW1a_sb = load("w1a", [128, 544], BF, eW1a)
            eW1b_sb = load("w1b", [128, 544], BF, eW1b)
            eW1c_sb = load("w1c", [16, 544], BF, eW1c)
            eW2m_sb = load("w2m", [128, 4, 128], BF, eW2m)
            eW2t_sb = load("w2t", [32, 128], BF, eW2t)
            cW1_sb = load("cw1", [128, 256], BF, cW1)
            cW2m_sb = load("cw2", [128, 2, 8], BF, cW2m)
            nW1m_sb = load("nw1", [128, 2, 4, 128], BF, nW1m)
            nW2m_sb = load("nw2", [128, 4, 128], BF, nW2m)
            eb1_sb = load("eb1", [128, 5], F32, eb1)
            eb2_sb = load("eb2", [128, 1], F32, eb2)
            cb1_sb = load("cb1", [128, 2], F32, cb1)
            nb1_sb = load("nb1", [128, 4], F32, nb1)
            nb2_sb = load("nb2", [128, 1], F32, nb2)

            iota_i = cp.tile([128, 128], I32, tag="iota_i")
            nc.gpsimd.iota(iota_i[:], pattern=[[1, 128]], base=0,
                           channel_multiplier=0)
            iota_f = cp.tile([128, 128], F32, tag="iota_f")
            nc.vector.tensor_copy(iota_f[:], iota_i[:])
            ident = cp.tile([128, 128], BF, tag="ident")
            make_identity(nc, ident[:])

            for w in range(WPC):
                # per-window PSUM accumulators (separate banks: one open
                # accumulation group per bank)
                aggp = pa.tile([128, 128], F32, tag="aggp")
                aggcp = pa.tile([128, 16], F32, tag="aggcp")

                for t4 in range(NT4):
                    b512 = NT4 * w + t4
                    isl = slice(32 * b512, 32 * b512 + 32)
                    # ---- gathers for this 512-edge tile ----
                    g_row = wp.tile([128, 1, 512], BF, tag="grow")
                    nc.gpsimd.dma_gather(
                        out_ap=g_row[:], in_ap=hW[:], idxs_ap=gidxr_sb[:, isl],
                        num_idxs=512, num_idxs_reg=512, elem_size=F,
                        transpose=True)
                    g_col = wp.tile([128, 1, 512], BF, tag="gcol")
                    nc.gpsimd.dma_gather(
                        out_ap=g_col[:], in_ap=hW[:], idxs_ap=gidxc_sb[:, isl],
                        num_idxs=512, num_idxs_reg=512, elem_size=F,
                        transpose=True)
                    c_row = wp.tile([128, 4, 64], F32, tag="crow")
                    nc.gpsimd.dma_gather(
                        out_ap=c_row[:], in_ap=coordp[:], idxs_ap=gidxr_sb[:, isl],
                        num_idxs=512, num_idxs_reg=512, elem_size=64,
                        transpose=False)
                    c_col = wp.tile([128, 4, 64], F32, tag="ccol")
                    nc.gpsimd.dma_gather(
                        out_ap=c_col[:], in_ap=coordp[:], idxs_ap=gidxc_sb[:, isl],
                        num_idxs=512, num_idxs_reg=512, elem_size=64,
                        transpose=False)

                    # ---- coord_diff & radial ----
                    cd = wp.tile([128, 4, CD], F32, tag="cd")
                    nc.vector.tensor_tensor(
                        out=cd[:], in0=c_row[:, :, 0:CD], in1=c_col[:, :, 0:CD],
                        op=OP.subtract)
                    t27 = wp.tile([128, 4, 27], F32, tag="t27")
                    cd4 = cd[:].rearrange("p t (c d) -> p t c d", c=3)
                    in0 = cd4.unsqueeze(3).broadcast_to([128, 4, 3, 3, 3])
                    in1 = cd4.unsqueeze(2).broadcast_to([128, 4, 3, 3, 3])
                    nc.vector.tensor_tensor(
                        out=t27[:].rearrange("p t (c f d) -> p t c f d", c=3, f=3),
                        in0=in0, in1=in1, op=OP.mult)
                    radial = wp.tile([128, 4, CD], F32, tag="radial")
                    nc.vector.tensor_reduce(
                        out=radial[:], in_=t27[:].rearrange("p t (x d) -> p t x d", d=3),
                        axis=mybir.AxisListType.X, op=OP.add)
                    rad_bf = wp.tile([128, 4, 16], BF, tag="radbf")
                    nc.vector.memset(rad_bf[:], 0)
                    nc.vector.tensor_copy(rad_bf[:, :, 0:CD], radial[:])
                    radT = wp.tile([16, 4, 128], BF, tag="radT")
                    rtp = pp.tile([16, 4, 128], BF, tag="tps")
                    for j in range(4):
                        nc.tensor.transpose(rtp[:, j, :], rad_bf[:, j, :], ident[:])
                    nc.vector.tensor_copy(radT[:], rtp[:])
                    # ---- edge MLP layer 1 ----
                    hid = []
                    for m in range(5):
                        mw = 128 if m < 4 else 32
                        msl = slice(128 * m, 128 * m + mw)
                        hp = pp.tile([mw, 512], F32, tag="mm512")
                        nc.tensor.matmul(hp[:], eW1a_sb[:, msl],
                                         g_row[:, 0, :], start=True, stop=False)
                        nc.tensor.matmul(hp[:], eW1b_sb[:, msl],
                                         g_col[:, 0, :], start=False, stop=False)
                        nc.tensor.matmul(
                            hp[:], eW1c_sb[:, msl],
                            radT[:].rearrange("p a b -> p (a b)"),
                            start=False, stop=True)
                        hs = wp.tile([mw, 512], BF, tag=f"hid{m}")
                        nc.scalar.activation(hs[:], hp[:], AF.Silu,
                                             bias=eb1_sb[0:mw, m:m + 1])
                        hid.append(hs)
                    # ---- edge MLP layer 2 ----
                    efp = pp.tile([128, 512], F32, tag="mm512")
                    for m in range(4):
                        nc.tensor.matmul(efp[:], eW2m_sb[:, m, :], hid[m][:],
                                         start=(m == 0), stop=False)
                    nc.tensor.matmul(efp[:], eW2t_sb[:], hid[4][:],
                                     start=False, stop=True)
                    ef = wp.tile([128, 512], BF, tag="ef")
                    nc.scalar.activation(ef[:], efp[:], AF.Silu,
                                         bias=eb2_sb[:, 0:1])
                    # ---- coord MLP ----
                    h2 = []
                    for m in range(2):
                        h2p = pp.tile([128, 512], F32, tag="mm512")
                        nc.tensor.matmul(h2p[:], cW1_sb[:, 128 * m:128 * m + 128],
                                         ef[:], start=True, stop=True)
                        h2s = wp.tile([128, 512], BF, tag=f"h2{m}")
                        nc.scalar.activation(h2s[:], h2p[:], AF.Silu,
                                             bias=cb1_sb[:, m:m + 1])
                        h2.append(h2s)
                    php = pp.tile([8, 512], F32, tag="ps2")
                    nc.tensor.matmul(php[:], cW2m_sb[:, 0, :], h2[0][:],
                                     start=True, stop=False)
                    nc.tensor.matmul(php[:], cW2m_sb[:, 1, :], h2[1][:],
                                     start=False, stop=True)
                    phT = wp.tile([8, 512], BF, tag="phT")
                    nc.vector.tensor_copy(phT[:], php[:])
                    # transpose phi -> [128, 4, 8]
                    phi = wp.tile([128, 4, 8], F32, tag="phi")
                    ptp = pp.tile([128, 4, 8], BF, tag="tps")
                    for j in range(4):
                        nc.tensor.transpose(ptp[:, j, :], phT[:, 128 * j:128 * j + 128],
                                            ident[0:8, 0:8])
                    nc.vector.tensor_copy(phi[:], ptp[:])
                    # trans = coord_diff * phi (broadcast over d)
                    trans = wp.tile([128, 4, CD], BF, tag="trans")
                    cdv = cd[:].rearrange("p t (c d) -> p t c d", c=3)
                    phv = phi[:, :, 0:3].unsqueeze(3).broadcast_to([128, 4, 3, 3])
                    nc.vector.tensor_tensor(
                        out=trans[:].rearrange("p t (c d) -> p t c d", c=3),
                        in0=cdv, in1=phv, op=OP.mult)
                    # ---- aggregation ----
                    etp = pp.tile([128, 4, 128], BF, tag="tps")
                    for j in range(4):
                        nc.tensor.transpose(etp[:, j, :], ef[:, 128 * j:128 * j + 128],
                                            ident[:])
                    eft = wp.tile([128, 4, 128], BF, tag="eft")
                    nc.vector.tensor_copy(eft[:], etp[:])
                    smat = wp.tile([128, 4, 128], BF, tag="smat")
                    c0 = NJ16 * w + 4 * t4
                    relv = rel_sb[:, c0:c0 + 4].unsqueeze(2).broadcast_to([128, 4, 128])
                    iov = iota_f[:].unsqueeze(1).broadcast_to([128, 4, 128])
                    nc.vector.tensor_tensor(out=smat[:], in0=relv, in1=iov,
                                            op=OP.is_equal)
                    for j in range(4):
                        g16 = 4 * t4 + j
                        nc.tensor.matmul(aggp[:, 0:128], smat[:, j, :], eft[:, j, :],
                                         start=(g16 == 0), stop=(g16 == NJ16 - 1),
                                         skip_group_check=True)
                        nc.tensor.matmul(aggcp[:, 0:9], smat[:, j, :],
                                         trans[:, j, :],
                                         start=(g16 == 0), stop=(g16 == NJ16 - 1),
                                         skip_group_check=True)

                # ---- node update for this window ----
                nsl = slice(128 * w, 128 * w + 128)
                agg_sb = wp.tile([128, 137], F32, tag="aggsb")
                nc.vector.tensor_copy(agg_sb[:, 0:128], aggp[:, 0:128])
                nc.vector.tensor_copy(agg_sb[:, 128:137], aggcp[:, 0:9])
                aggh_bf = wp.tile([128, 128], BF, tag="agghbf")
                nc.vector.tensor_copy(aggh_bf[:], agg_sb[:, 0:128])
                atp = pp.tile([128, 128], BF, tag="tps")
                nc.tensor.transpose(atp[:], aggh_bf[:], ident[:])
                aggT = wp.tile([128, 128], BF, tag="aggT")
                nc.vector.tensor_copy(aggT[:], atp[:])
                h3 = []
                for m in range(4):
                    h3p = pp.tile([128, 128], F32, tag="ps2")
                    nc.tensor.matmul(h3p[:], nW1m_sb[:, 0, m, :], hT_sb[:, nsl],
                                     start=True, stop=False)
                    nc.tensor.matmul(h3p[:], nW1m_sb[:, 1, m, :], aggT[:],
                                     start=False, stop=True)
                    h3s = wp.tile([128, 128], BF, tag=f"h3{m}")
                    nc.scalar.activation(h3s[:], h3p[:], AF.Silu,
                                         bias=nb1_sb[:, m:m + 1])
                    h3.append(h3s)
                h4p = pp.tile([128, 128], F32, tag="ps2")
                for m in range(4):
                    nc.tensor.matmul(h4p[:], nW2m_sb[:, m, :], h3[m][:],
                                     start=(m == 0), stop=(m == 3))
                h4s = wp.tile([128, 128], F32, tag="h4s")
                nc.scalar.activation(h4s[:], h4p[:], AF.Silu,
                                     bias=nb2_sb[:, 0:1])
                hout = wp.tile([128, 128], F32, tag="hout")
                nc.vector.tensor_tensor(out=hout[:], in0=h4s[:],
                                        in1=hTf_sb[:, nsl], op=OP.add)
                nc.sync.dma_start(houtT[:, nsl], hout[:])
                # coord update
                aggc = wp.tile([128, CD], F32, tag="aggc")
                nc.vector.tensor_scalar(
                    out=aggc[:], in0=agg_sb[:, 128:137],
                    scalar1=invdeg_sb[:, w:w + 1], scalar2=None, op0=OP.mult)
                nc.vector.tensor_scalar(
                    out=aggc[:], in0=aggc[:], scalar1=COORD_CLAMP,
                    scalar2=-COORD_CLAMP, op0=OP.min, op1=OP.max)
                co = wp.tile([128, CD], F32, tag="co")
                nc.vector.tensor_tensor(out=co[:], in0=aggc[:],
                                        in1=coordn_sb[:, w, :], op=OP.add)
                nc.sync.dma_start(coordo[:, w, :], co[:])

    nc.compile()
    return nc


# ---------------------------------------------------------------------------
# host-side prep / post
# ---------------------------------------------------------------------------

def _pack_windows(row, deg):
    """Greedy pack nodes (in order) into windows with <=NPW nodes and
    <=CAPE edges. Returns list of (node_start, n_nodes, edge_start, n_edges)
    where edge_start indexes the row-sorted edge order."""
    windows = []
    cum = np.concatenate([[0], np.cumsum(deg)])
    nstart = 0
    cur_nodes = 0
    cur_edges = 0
    for n in range(N):
        d = int(deg[n])
        if cur_nodes + 1 > NPW or cur_edges + d > CAPE:
            windows.append((nstart, cur_nodes, int(cum[nstart]), cur_edges))
            nstart = n
            cur_nodes = 0
            cur_edges = 0
        cur_nodes += 1
        cur_edges += d
    windows.append((nstart, cur_nodes, int(cum[nstart]), cur_edges))
    assert len(windows) <= NWIN, f"packing needs {len(windows)} windows > {NWIN}"
    while len(windows) < NWIN:
        windows.append((0, 0, 0, 0))
    return windows


def prepare(inputs):
    h = np.ascontiguousarray(np.asarray(inputs["h"], dtype=np.float32))
    ei = np.asarray(inputs["edge_index"])
    coord = np.asarray(inputs["coord"], dtype=np.float32)
    row = ei[0].astype(np.int64)
    col = ei[1].astype(np.int64)

    deg = np.bincount(row, minlength=N)
    order = np.argsort(row, kind="stable")
    windows = _pack_windows(row, deg)

    # edge slot arrays (global, then sliced per core)
    slot_row = np.zeros(NWIN * CAPE, dtype=np.int16)
    slot_col = np.zeros(NWIN * CAPE, dtype=np.int16)
    slot_rel = np.full(NWIN * CAPE, 255.0, dtype=np.float32)
    slot2node = np.full(NWIN * NPW, -1, dtype=np.int64)
    for w, (ns, nn, es, ne) in enumerate(windows):
        if ne:
            eidx = order[es:es + ne]
            sl = slice(w * CAPE, w * CAPE + ne)
            slot_row[sl] = row[eidx].astype(np.int16)
            slot_col[sl] = col[eidx].astype(np.int16)
            slot_rel[sl] = (row[eidx] - ns).astype(np.float32)
        if nn:
            slot2node[w * NPW:w * NPW + nn] = ns + np.arange(nn)

    hW = h.astype(BF16)
    coordp = np.zeros((N, 64), dtype=np.float32)
    coordp[:, :CD] = coord.reshape(N, CD)

    # weights
    def bf(x):
        return np.ascontiguousarray(np.asarray(x, dtype=np.float32)).astype(BF16)

    eW1 = np.asarray(inputs["eW1"], dtype=np.float32)
    eW1a = np.zeros((128, 544), np.float32); eW1a[:, :EH] = eW1[0:128]
    eW1b = np.zeros((128, 544), np.float32); eW1b[:, :EH] = eW1[128:256]
    eW1c = np.zeros((16, 544), np.float32); eW1c[:CD, :EH] = eW1[256:265]
    eW2 = np.asarray(inputs["eW2"], dtype=np.float32)
    eW2m = eW2[:512].reshape(4, 128, 128).transpose(1, 0, 2)
    eW2t = np.zeros((32, 128), np.float32); eW2t[:EH - 512] = eW2[512:EH]
    cW1 = np.asarray(inputs["cW1"], dtype=np.float32)
    cW2 = np.asarray(inputs["cW2"], dtype=np.float32)
    cW2m = np.zeros((128, 2, 8), np.float32)
    cW2m[:, :, 0:3] = cW2.reshape(2, 128, 3).transpose(1, 0, 2)
    nW1 = np.asarray(inputs["nW1"], dtype=np.float32)
    nW1m = nW1.reshape(2, 128, 4, 128).transpose(1, 0, 2, 3)
    nW2 = np.asarray(inputs["nW2"], dtype=np.float32)
    nW2m = nW2.reshape(4, 128, 128).transpose(1, 0, 2)

    def bias_fold(b, cols):
        z = np.zeros(128 * cols, np.float32)
        z[:b.shape[0]] = np.asarray(b, dtype=np.float32)
        return z.reshape(cols, 128).T.copy()

    eb1 = bias_fold(inputs["eb1"], 5)
    eb2 = bias_fold(inputs["eb2"], 1)
    cb1 = bias_fold(inputs["cb1"], 2)
    nb1 = bias_fold(inputs["nb1"], 4)
    nb2 = bias_fold(inputs["nb2"], 1)

    shared = {
        "hW": hW, "coordp": coordp,
        "eW1a": bf(eW1a), "eW1b": bf(eW1b), "eW1c": bf(eW1c),
        "eW2m": bf(eW2m), "eW2t": bf(eW2t),
        "cW1": bf(cW1), "cW2m": bf(cW2m),
        "nW1m": bf(nW1m), "nW2m": bf(nW2m),
        "eb1": eb1, "eb2": eb2, "cb1": cb1, "nb1": nb1, "nb2": nb2,
    }

    in_maps = []
    coord9 = coord.reshape(N, CD)
    invdeg_full = (1.0 / np.maximum(deg, 1)).astype(np.float32)
    for c in range(NCORES):
        esl = slice(c * EPC, (c + 1) * EPC)
        gr = np.tile(slot_row[esl].reshape(EPC // 16, 16).T, (8, 1)).copy()
        gc = np.tile(slot_col[esl].reshape(EPC // 16, 16).T, (8, 1)).copy()
        rl = slot_rel[esl].reshape(EPC // 128, 128).T.copy()
        s2n = slot2node[c * NSPC:(c + 1) * NSPC]
        valid = s2n >= 0
        idx0 = np.where(valid, s2n, 0)
        hT_c = (h[idx0] * valid[:, None]).T.copy()
        cn = (coord9[idx0] * valid[:, None]).reshape(WPC, NPW, CD)
        cn = np.ascontiguousarray(cn.transpose(1, 0, 2))
        ivd = (invdeg_full[idx0] * valid).reshape(WPC, NPW).T.copy()
        in_maps.append({
            **shared,
            "gidx_row": gr, "gidx_col": gc, "rel": rl,
            "hT": hT_c.astype(BF16), "hTf": hT_c.astype(np.float32),
            "coordn": cn.astype(np.float32), "invdeg": ivd.astype(np.float32),
        })

    meta = {"slot2node": slot2node}
    return in_maps, meta


def postprocess(results, meta):
    slot2node = meta["slot2node"]
    h_out = np.zeros((N, F), dtype=np.float32)
    coord_out = np.zeros((N, CD), dtype=np.float32)
    for c in range(NCORES):
        s2n = slot2node[c * NSPC:(c + 1) * NSPC]
        valid = s2n >= 0
        ho = results[c]["houtT"].T            # [NSPC, F]
        co = np.ascontiguousarray(
            results[c]["coordo"].transpose(1, 0, 2)).reshape(NSPC, CD)
        h_out[s2n[valid]] = ho[valid]
        coord_out[s2n[valid]] = co[valid]
    return h_out, coord_out.reshape(N, C, D)


# ---------------------------------------------------------------------------
# entry point
# ---------------------------------------------------------------------------

_NC_CACHE = None


def _get_nc():
    global _NC_CACHE
    if _NC_CACHE is None:
        _NC_CACHE = build_nc()
    return _NC_CACHE


def run(inputs, **run_kwargs):
    """Returns ((h_out, coord_out), BassKernelResults)."""
    in_maps, meta = prepare(inputs)
    nc = _get_nc()
    res = run_bass_kernel_spmd(nc, in_maps, core_ids=list(range(NCORES)),
                               **run_kwargs)
    return postprocess(res.results, meta), res


def kernel(**inputs):
    out, _ = run(inputs)
    return out
